# revision 85
# baseline (speedup 1.0000x reference)
"""Batch Child-Sum TreeLSTM on 8 NeuronCores — v4 bf16-split + PE-scatter.

v4 over v3 (~1.35ms -> ~1.21ms): (1) the gate GEMM writes f into its OWN
PSUM tile so c_last/gdst/gt-f depend only on the f third of the burst
(whole-tile deps made them wait for the full burst: semaphore increments
land at group stops); (2) h_pre is emitted AFTER h_last so it cannot
steal the DVE slot from the chain-critical hs3 ops (mid-block step went
12.4us with p90 46us at boundaries); (3) block boundaries: the block's
own h-sum into step 8j+8 is computed elementwise from H_blk + mk0 row 0
(off the ht_build/pe_hside chain), and c_pre/c_last extend across the
boundary (prev-block taus 0..6 pre-computed at block end with the mk0
mask, tau 7 via the PSUM G cols at l==0), dropping both 8j+8 scatter
chunks from the critical path.

Data-parallel over batch (16 rows/core).  The O(S^2) masked prefix work is
split into per-block (T=8 steps) scatter passes accumulating fc_acc/hs_acc
for future steps; the per-step chain handles in-block tails (split into an
off-chain pre-tail over taus<l-1 plus a tiny on-chain last term), the gate
GEMM, and the state update.

Gate GEMM (v3): fp32 matmuls serialize LDW+MM at ~517 ns/pair on TRN2
(no FWL, no background weight buffer for fp32).  Instead W and hs2 are
split hi+lo bf16 and three products accumulate in PSUM (W_hi@[h_hi|h_lo]
fused-N + W_lo@h_hi onto the lo cols): 96 bf16 pairs/step ~= 4.0 us vs
24.8 us fp32 (split residual ~2^-17; end-to-end rel err 2.2e-4).

h-side scatter (v3): hs_acc[t] += sum_tau m*H is a per-batch tau-
contraction = PE matmul with a host-built block-diagonal bf16 mask
L[(tau,b),(t,b)], H transposed per block into [tau*16+b, h] bf16 hi/lo
via 4 PE transposes (+c-major staging copy; transpose outputs must land
at PSUM partition 0), and the result transposed back and added into
hs_acc.  Replaces ~700 us of gpsimd/DVE elementwise per run.

Numerics: state C and all accumulators stay fp32 (c reaches ~5e8 and the
recurrence amplifies; plain bf16 on h/C/weights fails the 2e-2 gate).
bf16 hi+lo splits (~17 mantissa bits) are safe everywhere they are used.

Layout: partition p = h % 128; free cols = c*16+b (c = h//128, b =
batch-in-core).  Masks are host-exact, DMA-broadcast via stride-0
partition APs.  GPSIMD cannot touch PSUM (BIR verifier), and DVE reads
at most ONE PSUM operand per instruction (NCC_IBVF027) — hence the
gt = (zx + z_hi) + z_lo ordering.  Engine rates measured in isolation:
Act 0.92, DVE 0.77, GpSimd 0.39 free-els/ns (fp32, AP-shape-insensitive);
in-kernel DVE drops ~2x under multi-engine SBUF contention.
"""

import numpy as np

import concourse.bass as bass
import concourse.bacc as bacc
import concourse.tile as tile
from concourse import mybir

F32 = mybir.dt.float32
BF16 = mybir.dt.bfloat16
AF = mybir.ActivationFunctionType
ALU = mybir.AluOpType

NCORES = 8
B, S, E, H, V, L = 128, 64, 300, 512, 100000, 5
BC = B // NCORES   # 16 batch rows per core
EK = 304           # padded E+bias rows (300 data + 1 ones + 3 zero)
KC = [128, 128, 48]
T = 8              # block size
NBLK = S // T      # 8 blocks
# bulk-mask row offsets per block j (rows of wmk): block j has 56-8j rows
WMK_OFF = [0]
for _j in range(NBLK - 1):
    WMK_OFF.append(WMK_OFF[-1] + (S - 8 * _j - 8))
WMK_ROWS = WMK_OFF[-1] + 8   # 224

LINEARIZE = False
_prog_cache = {}

# h-side scatter chunks (j, t0, nt) in device emission order + L-mask col offs
HCHUNKS = []
for _j in range(NBLK - 1):
    HCHUNKS.append((_j, 8 * _j + 8, 1))
    HCHUNKS.append((_j, 8 * _j + 9, 1))
    HCHUNKS.append((_j, 8 * _j + 10, 6))
    for _t0 in range(8 * _j + 16, S, 8):
        HCHUNKS.append((_j, _t0, 8))
HOFF = {}
_off = 0
for (_j, _t0, _nt) in HCHUNKS:
    HOFF[(_j, _t0)] = _off
    _off += _nt * 16
LCOLS = _off


def _ap(t_ap, dims, doff=0):
    return bass.AP(tensor=t_ap.tensor, offset=t_ap.offset + doff,
                   ap=[t_ap.ap[0], *dims])


def _build_program(repeat=1):
    key = ("nc", repeat)
    if key in _prog_cache:
        return _prog_cache[key]

    nc = bacc.Bacc(None, target_bir_lowering=False, debug=False)

    xet_d = nc.declare_dram_parameter("xet", [EK, S * BC], F32, isOutput=False)
    wxt_d = nc.declare_dram_parameter("wxt", [EK, 3 * H], F32, isOutput=False)
    whi_d = nc.declare_dram_parameter("wht_hi", [H, 3 * H], BF16, isOutput=False)
    wlo_d = nc.declare_dram_parameter("wht_lo", [H, 3 * H], BF16, isOutput=False)
    wout_d = nc.declare_dram_parameter("wout", [H, L], F32, isOutput=False)
    bout_d = nc.declare_dram_parameter("bout_rep", [BC, L], F32, isOutput=False)
    wmt_d = nc.declare_dram_parameter("wmt", [NBLK * 7, T * BC], F32,
                                      isOutput=False)
    wmk_d = nc.declare_dram_parameter("wmk", [WMK_ROWS, T * BC], F32,
                                      isOutput=False)
    lmk_d = nc.declare_dram_parameter("lmk", [128, LCOLS], BF16,
                                      isOutput=False)
    ident_d = nc.declare_dram_parameter("ident", [128, 128], F32,
                                        isOutput=False)
    out_d = nc.declare_dram_parameter("out", [BC, L], F32, isOutput=True)

    with tile.TileContext(nc, linearize=LINEARIZE) as tc:
        with (
            tc.tile_pool(name="singles", bufs=1) as sg,
        ):
            # -------- persistent --------
            zx_all = sg.tile([128, S, 192], F32)   # x-side gates (i,f,u)
            whi_s = sg.tile([128, 4, 3 * H], BF16)
            wlo_s = sg.tile([128, 4, 3 * H], BF16)
            wout_s = sg.tile([128, 4, L], F32)
            bout_s = sg.tile([BC, L], F32)
            fc_acc = sg.tile([128, S, 64], F32)
            hs_acc = sg.tile([128, S, 64], F32)
            ident_s = sg.tile([128, 128], F32)
            nc.gpsimd.dma_start(out=ident_s[:], in_=ident_d[:])

            nc.gpsimd.dma_start(
                out=whi_s[:],
                in_=whi_d[:].rearrange("(k1 p) n -> p k1 n", p=128))
            nc.gpsimd.dma_start(
                out=wlo_s[:],
                in_=wlo_d[:].rearrange("(k1 p) n -> p k1 n", p=128))
            nc.gpsimd.dma_start(
                out=wout_s[:],
                in_=wout_d[:].rearrange("(k1 p) n -> p k1 n", p=128))
            nc.gpsimd.dma_start(out=bout_s[:], in_=bout_d[:])

            # -------- phase 1: x-side GEMM --------
            with (
                tc.tile_pool(name="xw", bufs=1) as xw,
                tc.tile_pool(name="ph1p", bufs=2, space="PSUM") as ph1p,
            ):
                wxt_c = [xw.tile([128, 3 * H], F32, name=f"wxt{k}",
                                 tag=f"wxt{k}") for k in range(3)]
                xet_c = [xw.tile([128, S * BC], F32, name=f"xet{k}",
                                 tag=f"xet{k}") for k in range(3)]
                for k1 in range(3):
                    cnt = KC[k1]
                    nc.gpsimd.dma_start(out=wxt_c[k1][0:cnt, :],
                                        in_=wxt_d[k1 * 128:k1 * 128 + cnt, :])
                    nc.gpsimd.dma_start(out=xet_c[k1][0:cnt, :],
                                        in_=xet_d[k1 * 128:k1 * 128 + cnt, :])
                for nh in range(2):
                    for g in range(3):
                        for c in range(4):
                            zxp = ph1p.tile([128, 512], F32, tag="zxp")
                            for k1 in range(3):
                                cnt = KC[k1]
                                nc.tensor.matmul(
                                    out=zxp[:],
                                    lhsT=wxt_c[k1][0:cnt,
                                                   512 * g + 128 * c:
                                                   512 * g + 128 * c + 128],
                                    rhs=xet_c[k1][0:cnt,
                                                  512 * nh:512 * nh + 512],
                                    start=(k1 == 0), stop=(k1 == 2))
                            # psum cols (t, b) -> zx_all[:, t, g*64+c*16+b]
                            dst = _ap(zx_all[:], [[192, 32], [1, 16]],
                                      doff=(nh * 32) * 192 + g * 64 + c * 16)
                            nc.scalar.activation(
                                out=dst,
                                in_=zxp[:].rearrange("p (t b) -> p t b", b=16),
                                func=AF.Copy)

            # -------- phase 2 --------
            with (
                tc.tile_pool(name="st", bufs=2) as stp,     # block state
                tc.tile_pool(name="sm", bufs=3) as sm,      # small per-step
                tc.tile_pool(name="mk", bufs=2) as mkp,     # bulk masks
                tc.tile_pool(name="mt", bufs=2) as mtp,     # tail masks
                tc.tile_pool(name="sp", bufs=2) as spp,     # scatter scratch
                tc.tile_pool(name="hts", bufs=2) as hts,    # H^T bf16 hi/lo
                tc.tile_pool(name="lmp", bufs=2) as lmp,    # L masks
                tc.tile_pool(name="zsp", bufs=1) as zsp,    # hside sbuf stage
                tc.tile_pool(name="zp", bufs=2, space="PSUM") as zp,
                tc.tile_pool(name="htp", bufs=1, space="PSUM") as htp,
                tc.tile_pool(name="sc", bufs=2, space="PSUM") as scp,
                tc.tile_pool(name="op", bufs=1, space="PSUM") as opp,
            ):
                env = dict(locals())
                for _rep in range(repeat):
                    _phase2(nc, env)

    nc.finalize()
    _prog_cache[key] = nc
    return nc


def _phase2(nc, env):
    zx_all = env["zx_all"]; whi_s = env["whi_s"]; wlo_s = env["wlo_s"]
    wout_s = env["wout_s"]
    bout_s = env["bout_s"]; fc_acc = env["fc_acc"]; hs_acc = env["hs_acc"]
    wmt_d = env["wmt_d"]; wmk_d = env["wmk_d"]; out_d = env["out_d"]
    lmk_d = env["lmk_d"]; ident_s = env["ident_s"]
    stp = env["stp"]; sm = env["sm"]; mkp = env["mkp"]; mtp = env["mtp"]
    spp = env["spp"]; zp = env["zp"]; opp = env["opp"]
    hts = env["hts"]; lmp = env["lmp"]; zsp = env["zsp"]
    htp = env["htp"]; scp = env["scp"]; tc = env["tc"]

    nc.vector.memset(fc_acc[:], 0.0)
    nc.gpsimd.memset(hs_acc[:], 0.0)

    state = {"hs3": None, "prev_H": None, "fct2": None, "fct2_pre": None,
             "hs_pre": None, "prev_G": None, "pending_cside": []}
    scatter_q = []  # deferred far-chunk thunks of the previous block

    def load_tail_mask(j):
        tm = mtp.tile([128, 7, T * BC], F32, tag="tm")
        src = bass.AP(tensor=wmt_d[:].tensor,
                      offset=wmt_d[:].offset + (j * 7) * (T * BC),
                      ap=[[0, 128], [T * BC, 7], [1, T * BC]])
        nc.gpsimd.dma_start(out=tm[:], in_=src)
        return tm

    def load_bulk_mask(j, t0, nt, tag):
        mk = mkp.tile([128, 8, T * BC], F32, tag=tag)
        row = WMK_OFF[j] + (t0 - 8 * j - 8)
        src = bass.AP(tensor=wmk_d[:].tensor,
                      offset=wmk_d[:].offset + row * (T * BC),
                      ap=[[0, 128], [T * BC, nt], [1, T * BC]])
        nc.gpsimd.dma_start(
            out=_ap(mk[:], [[T * BC, nt], [1, T * BC]]), in_=src)
        return mk

    def ht_build(H_blk):
        """Transpose the block's H into H^T bf16 hi/lo tiles [tau*16+b, h]."""
        ht_ps, ht_hi, ht_lo = state["ht"]
        # stage c-major so each per-c transpose input is contiguous
        st_h = zsp.tile([128, 4, 128], F32, tag="sth")
        nc.gpsimd.tensor_copy(
            out=_ap(st_h[:], [[16, 8], [128, 4], [1, 16]]),
            in_=_ap(H_blk[:], [[64, 8], [16, 4], [1, 16]]))
        for c in range(4):
            nc.tensor.transpose(
                out=ht_ps[:, c * 128:c * 128 + 128],
                in_=st_h[:, c, :], identity=ident_s[:])
            nc.scalar.activation(
                out=ht_hi[:, c, :],
                in_=ht_ps[:, c * 128:c * 128 + 128], func=AF.Copy)
            nc.vector.tensor_sub(
                out=ht_lo[:, c, :],
                in0=ht_ps[:, c * 128:c * 128 + 128],
                in1=ht_hi[:, c, :])

    def pe_hside(t0, nt, off, ht, prio=0):
        """hs_acc[t0:t0+nt] += masked sum of block H via PE matmul."""
        if prio:
            with tc.high_priority(offset=prio):
                return _pe_hside(t0, nt, off, ht)
        return _pe_hside(t0, nt, off, ht)

    def _pe_hside(t0, nt, off, ht):
        ht_hi, ht_lo = ht
        m = nt * 16
        lt = lmp.tile([128, 128], BF16, tag="lm")
        nc.gpsimd.dma_start(out=lt[:, 0:m], in_=lmk_d[:, off:off + m])
        zT = scp.tile([128, 512], F32, tag="sc")
        for c in range(4):
            nc.tensor.matmul(out=zT[0:m, c * 128:c * 128 + 128],
                             lhsT=lt[:, 0:m], rhs=ht_hi[:, c, :],
                             start=True, stop=False, skip_group_check=True)
            nc.tensor.matmul(out=zT[0:m, c * 128:c * 128 + 128],
                             lhsT=lt[:, 0:m], rhs=ht_lo[:, c, :],
                             start=False, stop=True, skip_group_check=True)
        zs = zsp.tile([128, 512], F32, tag="zs")
        nc.scalar.activation(out=zs[0:m, :], in_=zT[0:m, :], func=AF.Copy)
        tb = scp.tile([128, 512], F32, tag="sc")
        for c in range(4):
            nc.tensor.transpose(out=tb[:, c * m:c * m + m],
                                in_=zs[0:m, c * 128:c * 128 + 128],
                                identity=ident_s[0:m, 0:m])
            nc.vector.tensor_add(
                out=_ap(hs_acc[:], [[64, nt], [1, 16]],
                        doff=t0 * 64 + c * 16),
                in0=_ap(hs_acc[:], [[64, nt], [1, 16]],
                        doff=t0 * 64 + c * 16),
                in1=_ap(tb[:], [[16, nt], [1, 16]], doff=c * m))

    def scatter_chunk(G_blk, C_blk, H_blk, mk_ap, t0, nt, hs_first,
                      h_only=False, c_only=False):
        _scatter_chunk(G_blk, C_blk, H_blk, mk_ap, t0, nt)

    def _scatter_chunk(G_blk, C_blk, H_blk, mk_ap, t0, nt):
        """Contributions of block cols to t in [t0, t0+nt)."""
        s1 = spp.tile([128, T, 8 * 64], F32, tag="s1")
        m_ap = bass.AP(tensor=mk_ap.tensor, offset=mk_ap.offset,
                       ap=[mk_ap.ap[0],
                           [T * BC, nt], [16, 8], [0, 4], [1, 16]])
        d3 = [[512, nt], [64, 8], [1, 64]]
        d4 = [[512, nt], [64, 8], [16, 4], [1, 16]]

        def cside():
            nc.vector.tensor_add(
                out=_ap(s1[:], d3),
                in0=_ap(G_blk[:], [[0, nt], [64, 8], [1, 64]]),
                in1=_ap(zx_all[:], [[192, nt], [0, 8], [1, 64]],
                        doff=t0 * 192 + 64))
            nc.scalar.activation(out=s1[0:128, 0:nt, :],
                                 in_=s1[0:128, 0:nt, :], func=AF.Sigmoid)
            nc.vector.tensor_mul(
                out=_ap(s1[:], d3),
                in0=_ap(s1[:], d3),
                in1=_ap(C_blk[:], [[0, nt], [64, 8], [1, 64]]))
            nc.gpsimd.tensor_mul(
                out=_ap(s1[:], d4), in0=_ap(s1[:], d4), in1=m_ap)
            nc.vector.tensor_add(
                out=_ap(s1[:], [[512, nt], [64, 4], [1, 64]]),
                in0=_ap(s1[:], [[512, nt], [64, 4], [1, 64]]),
                in1=_ap(s1[:], [[512, nt], [64, 4], [1, 64]], doff=256))
            nc.vector.tensor_add(
                out=_ap(s1[:], [[512, nt], [64, 2], [1, 64]]),
                in0=_ap(s1[:], [[512, nt], [64, 2], [1, 64]]),
                in1=_ap(s1[:], [[512, nt], [64, 2], [1, 64]], doff=128))
            nc.gpsimd.tensor_add(
                out=_ap(s1[:], [[512, nt], [1, 64]]),
                in0=_ap(s1[:], [[512, nt], [1, 64]]),
                in1=_ap(s1[:], [[512, nt], [1, 64]], doff=64))
            nc.gpsimd.tensor_add(
                out=_ap(fc_acc[:], [[64, nt], [1, 64]], doff=t0 * 64),
                in0=_ap(fc_acc[:], [[64, nt], [1, 64]], doff=t0 * 64),
                in1=_ap(s1[:], [[512, nt], [1, 64]]))

        cside()

    def h_tail_boundary(j, t, mk0, H_blk):
        """hs2 for step t+1 = hs_acc[t+1] (far) + full-block masked sum.

        Keeps the block-boundary off the ht_build/pe_hside chain: the
        block's own contribution to step 8j+8 is computed elementwise
        from H_blk with mk0 row 0 instead of a PE scatter chunk.
        """
        hs2n = sm.tile([128, 4, 32], F32, tag="hs2")
        st2 = sm.tile([128, T, 64], F32, tag="st2")
        nc.gpsimd.tensor_mul(
            out=_ap(st2[:], [[64, 8], [16, 4], [1, 16]]),
            in0=_ap(H_blk[:], [[64, 8], [16, 4], [1, 16]]),
            in1=_ap(mk0[:], [[16, 8], [0, 4], [1, 16]]))
        hsr = sm.tile([128, 64], F32, tag="hsr")
        nc.vector.tensor_reduce(
            out=hsr[:], in_=_ap(st2[:], [[1, 64], [64, 8]]),
            axis=mybir.AxisListType.X, op=ALU.add)
        nc.gpsimd.tensor_add(
            out=_ap(hs2n[:], [[32, 4], [1, 16]]),
            in0=hsr[:].rearrange("p (c b) -> p c b", b=16),
            in1=hs_acc[:, t + 1, :].rearrange("p (c b) -> p c b", b=16))
        nc.gpsimd.tensor_copy(
            out=_ap(hs2n[:], [[32, 4], [1, 16]], doff=16),
            in_=H_blk[:, T - 1, :].rearrange("p (c b) -> p c b", b=16))
        hs3n = sm.tile([128, 4, 64], BF16, tag="hs3")
        nc.scalar.activation(out=hs3n[:, :, 0:32], in_=hs2n[:], func=AF.Copy)
        nc.vector.tensor_sub(out=hs3n[:, :, 32:64], in0=hs2n[:],
                             in1=hs3n[:, :, 0:32])
        state["hs3"] = hs3n

    def h_pre(j, l, t, tm, H_blk):
        """h_sum pre-tail for step t+1: taus 0..l-1 plus hs_acc[t+1]."""
        hsp = sm.tile([128, 4, 16], F32, tag="hsp")
        if l > 0:
            st2 = sm.tile([128, T, 64], F32, tag="st2")
            nc.gpsimd.tensor_mul(
                out=_ap(st2[:], [[64, l], [16, 4], [1, 16]]),
                in0=_ap(H_blk[:], [[64, l], [16, 4], [1, 16]]),
                in1=_ap(tm[:], [[16, l], [0, 4], [1, 16]],
                        doff=l * (T * BC)))
            hsr = sm.tile([128, 64], F32, tag="hsr")
            nc.vector.tensor_reduce(
                out=hsr[:], in_=_ap(st2[:], [[1, 64], [64, l]]),
                axis=mybir.AxisListType.X, op=ALU.add)
            nc.gpsimd.tensor_add(
                out=_ap(hsp[:], [[16, 4], [1, 16]]),
                in0=hsr[:].rearrange("p (c b) -> p c b", b=16),
                in1=hs_acc[:, t + 1, :].rearrange("p (c b) -> p c b", b=16))
        else:
            nc.gpsimd.tensor_copy(
                out=_ap(hsp[:], [[16, 4], [1, 16]]),
                in_=hs_acc[:, t + 1, :].rearrange("p (c b) -> p c b", b=16))
        state["hs_pre"] = hsp

    def h_last(j, l, t, tm, H_blk):
        """Finish hs2/hs3 for step t+1 with the tau=l last term."""
        hs2n = sm.tile([128, 4, 32], F32, tag="hs2")
        hlm = sm.tile([128, 64], F32, tag="hlm")
        nc.gpsimd.tensor_mul(
            out=_ap(hlm[:], [[16, 4], [1, 16]]),
            in0=H_blk[:, l, :].rearrange("p (c b) -> p c b", b=16),
            in1=_ap(tm[:], [[0, 4], [1, 16]],
                    doff=l * (T * BC) + l * 16))
        nc.vector.tensor_add(
            out=_ap(hs2n[:], [[32, 4], [1, 16]]),
            in0=_ap(state["hs_pre"][:], [[16, 4], [1, 16]]),
            in1=hlm[:].rearrange("p (c b) -> p c b", b=16))
        nc.gpsimd.tensor_copy(
            out=_ap(hs2n[:], [[32, 4], [1, 16]], doff=16),
            in_=H_blk[:, l, :].rearrange("p (c b) -> p c b", b=16))
        hs3n = sm.tile([128, 4, 64], BF16, tag="hs3")
        nc.scalar.activation(out=hs3n[:, :, 0:32], in_=hs2n[:], func=AF.Copy)
        nc.vector.tensor_sub(out=hs3n[:, :, 32:64], in0=hs2n[:],
                             in1=hs3n[:, :, 0:32])
        state["hs3"] = hs3n

    def c_pre(nl, tnext, mask_ap, G_blk, C_blk):
        """C-side pre-tail for step tnext: taus 0..nl-1 plus fc_acc."""
        st = zsp.tile([128, T, 64], F32, tag="st")
        nc.vector.tensor_add(
            out=_ap(st[:], [[64, nl], [1, 64]]),
            in0=G_blk[:, 0:nl, :],
            in1=_ap(zx_all[:], [[0, nl], [1, 64]],
                    doff=tnext * 192 + 64))
        nc.scalar.activation(out=st[:, 0:nl, :], in_=st[:, 0:nl, :],
                             func=AF.Sigmoid)
        nc.vector.tensor_mul(out=st[:, 0:nl, :], in0=st[:, 0:nl, :],
                             in1=C_blk[:, 0:nl, :])
        nc.vector.tensor_mul(
            out=_ap(st[:], [[64, nl], [16, 4], [1, 16]]),
            in0=_ap(st[:], [[64, nl], [16, 4], [1, 16]]),
            in1=mask_ap)
        fct = sm.tile([128, 64], F32, tag="fct")
        nc.vector.tensor_reduce(
            out=fct[:], in_=_ap(st[:], [[1, 64], [64, nl]]),
            axis=mybir.AxisListType.X, op=ALU.add)
        fct2p = sm.tile([128, 64], F32, tag="fct2p")
        nc.vector.tensor_add(out=fct2p[:], in0=fct[:],
                             in1=fc_acc[:, tnext, :])
        state["fct2_pre"] = fct2p

    def c_tau_scatter(j, l, t, tm, G_blk, C_blk):
        """Scatter tau=l-1's C-side contribution to steps t+1..8j+7.

        Emitted once per tau right after its G lands, replacing the
        per-step c_pre recompute (which re-read every earlier tau each
        step and needed a reduce); fct2 then comes straight from fc_acc.
        """
        nk = T - 1 - l
        cs = zsp.tile([128, 6, 64], F32, tag="cts")
        nc.vector.tensor_add(
            out=_ap(cs[:], [[64, nk], [1, 64]]),
            in0=_ap(G_blk[:], [[0, nk], [1, 64]], doff=(l - 1) * 64),
            in1=_ap(zx_all[:], [[192, nk], [1, 64]],
                    doff=(t + 1) * 192 + 64))
        nc.scalar.activation(out=_ap(cs[:], [[64, nk], [1, 64]]),
                             in_=_ap(cs[:], [[64, nk], [1, 64]]),
                             func=AF.Sigmoid)
        nc.vector.tensor_mul(
            out=_ap(cs[:], [[64, nk], [1, 64]]),
            in0=_ap(cs[:], [[64, nk], [1, 64]]),
            in1=_ap(C_blk[:], [[0, nk], [1, 64]], doff=(l - 1) * 64))
        nc.gpsimd.tensor_mul(
            out=_ap(cs[:], [[64, nk], [16, 4], [1, 16]]),
            in0=_ap(cs[:], [[64, nk], [16, 4], [1, 16]]),
            in1=_ap(tm[:], [[T * BC, nk], [0, 4], [1, 16]],
                    doff=l * (T * BC) + (l - 1) * 16))
        nc.vector.tensor_add(
            out=_ap(fc_acc[:], [[64, nk], [1, 64]], doff=(t + 1) * 64),
            in0=_ap(fc_acc[:], [[64, nk], [1, 64]], doff=(t + 1) * 64),
            in1=_ap(cs[:], [[64, nk], [1, 64]]))

    def c_last(t, zpsF, C_col, mask_ap):
        """fct2 for step t = pre-tail + last term (fresh G from PSUM)."""
        stl = sm.tile([128, 64], F32, tag="stl")
        nc.vector.tensor_add(
            out=stl[:], in0=zx_all[:, t, 64:128],
            in1=_ap(zpsF[:], [[64, 4], [1, 16]], doff=16))
        nc.vector.tensor_add(
            out=stl[:], in0=stl[:],
            in1=_ap(zpsF[:], [[64, 4], [1, 16]], doff=48))
        nc.scalar.activation(out=stl[:], in_=stl[:], func=AF.Sigmoid)
        nc.vector.tensor_mul(out=stl[:], in0=stl[:], in1=C_col)
        nc.gpsimd.tensor_mul(
            out=_ap(stl[:], [[16, 4], [1, 16]]),
            in0=_ap(stl[:], [[16, 4], [1, 16]]),
            in1=mask_ap)
        fct2 = sm.tile([128, 64], F32, tag="fct2")
        pre = state["fct2_pre"]
        nc.vector.tensor_add(
            out=fct2[:], in0=stl[:],
            in1=(fc_acc[:, t, :] if pre is None else pre[:]))
        state["fct2"] = fct2[:]

    for j in range(NBLK):
        G_blk = stp.tile([128, T, 64], F32, tag="G")
        C_blk = stp.tile([128, T, 64], F32, tag="C")
        H_blk = stp.tile([128, T, 64], F32, tag="H")
        tm = load_tail_mask(j)
        mk0 = (load_bulk_mask(j, 8 * j + 8, 8, tag="mk0")
               if j < NBLK - 1 else None)
        if j < NBLK - 1:
            ht_ps = htp.tile([128, 512], F32, tag="htp", name="ht_ps")
            ht_hi = hts.tile([128, 4, 128], BF16, tag="hthi", name="ht_hi")
            ht_lo = hts.tile([128, 4, 128], BF16, tag="htlo", name="ht_lo")
            state["ht"] = (ht_ps, ht_hi, ht_lo)

        for l in range(T):
            t = 8 * j + l
            zxs = zx_all[:, t, :]
            ga = sm.tile([128, 192], F32, tag="ga")
            if l == 0 and j == 0:
                h_pre(j, 0, 0, tm, H_blk)
            if t == 0:
                gt = sm.tile([128, 192], F32, tag="gt")
                nc.gpsimd.tensor_copy(out=gt[:], in_=zxs)
            else:
                hs3 = state["hs3"]
                # bf16 3-pass split: W_hi@[h_hi|h_lo] fused-N + W_lo@h_hi
                # (accumulated onto the lo cols). f gets its OWN psum tile
                # so c_last/gdst/gt-f wait only on the f MMs (first third
                # of the burst), not the whole-tile dependency.
                zpsF = zp.tile([128, 256], F32, tag="zf")
                zpsIU = zp.tile([128, 256], F32, tag="zu")
                for g, wof, ztile, base, nn in (
                        (1, 512, zpsF, 0, 32), (0, 0, zpsIU, 0, 16),
                        (2, 1024, zpsIU, 128, 16)):
                    for c in range(4):
                        o0 = base + 2 * nn * c
                        for k1 in range(4):
                            if nn == 16:
                                rfused = _ap(hs3[:, k1, :], [[32, 2], [1, 16]])
                                rhi = hs3[:, k1, 0:16]
                            else:
                                rfused = hs3[:, k1, 0:64]
                                rhi = hs3[:, k1, 0:32]
                            nc.tensor.matmul(
                                out=ztile[:, o0:o0 + 2 * nn],
                                lhsT=whi_s[:, k1,
                                           wof + 128 * c:wof + 128 * c + 128],
                                rhs=rfused,
                                start=(k1 == 0), stop=False,
                                skip_group_check=True)
                            nc.tensor.matmul(
                                out=ztile[:, o0 + nn:o0 + 2 * nn],
                                lhsT=wlo_s[:, k1,
                                           wof + 128 * c:wof + 128 * c + 128],
                                rhs=rhi,
                                start=False, stop=(k1 == 3),
                                skip_group_check=True)
                # fct2 for this step straight from the f-gate PSUM cols
                if l > 0:
                    c_last(t, zpsF, C_blk[:, l - 1, :],
                           _ap(tm[:], [[0, 4], [1, 16]],
                               doff=(l - 1) * (T * BC) + (l - 1) * 16))
                else:
                    # boundary: last term is tau=7 of the previous block
                    c_last(t, zpsF, state["prev_C"][:, T - 1, :],
                           _ap(state["prev_mk0"][:], [[0, 4], [1, 16]],
                               doff=(T - 1) * 16))
                    state["fct2_pre"] = None
                # G col of step t-1 = hp products (hi cols + lo cols);
                # feeds only c_pre/scatter, off the serial chain
                gdst = (G_blk[:, l - 1, :] if l > 0
                        else state["prev_G"][:, T - 1, :])
                nc.scalar.activation(
                    out=gdst,
                    in_=_ap(zpsF[:], [[64, 4], [1, 16]], doff=16),
                    func=AF.Copy)
                nc.vector.tensor_add(
                    out=gdst, in0=gdst,
                    in1=_ap(zpsF[:], [[64, 4], [1, 16]], doff=48))
                if l == 0:
                    for th in state["pending_cside"]:
                        th()
                    state["pending_cside"] = []
                # gates: gt = (z_hi + zx) + z_lo  (i | f | u); one PSUM
                # operand per DVE instruction (walrus NCC_IBVF027)
                gt = sm.tile([128, 192], F32, tag="gt")
                nc.vector.tensor_add(
                    out=gt[:, 64:128], in0=zxs[:, 64:128],
                    in1=_ap(zpsF[:], [[64, 4], [1, 16]], doff=0))
                nc.vector.tensor_add(
                    out=gt[:, 64:128], in0=gt[:, 64:128],
                    in1=_ap(zpsF[:], [[64, 4], [1, 16]], doff=32))
                nc.vector.tensor_add(
                    out=gt[:, 0:64], in0=zxs[:, 0:64],
                    in1=_ap(zpsIU[:], [[32, 4], [1, 16]], doff=0))
                nc.vector.tensor_add(
                    out=gt[:, 128:192], in0=zxs[:, 128:192],
                    in1=_ap(zpsIU[:], [[32, 4], [1, 16]], doff=128))
                nc.vector.tensor_add(
                    out=gt[:, 0:64], in0=gt[:, 0:64],
                    in1=_ap(zpsIU[:], [[32, 4], [1, 16]], doff=16))
                nc.vector.tensor_add(
                    out=gt[:, 128:192], in0=gt[:, 128:192],
                    in1=_ap(zpsIU[:], [[32, 4], [1, 16]], doff=128 + 16))
                # scatter tau=l-1 into the remaining in-block steps
                # (off-chain; uses fresh G[l-1])
                if 0 < l < T - 1:
                    c_tau_scatter(j, l, t, tm, G_blk, C_blk)
            nc.scalar.activation(out=ga[:, 0:128], in_=gt[:, 0:128],
                                 func=AF.Sigmoid)
            nc.scalar.activation(out=ga[:, 128:192], in_=gt[:, 128:192],
                                 func=AF.Tanh)

            # ---- c, h ----  (fct2 = pre-tail + last term, set by c_last)
            fct2 = state["fct2"] if t > 0 else None
            ctmp = sm.tile([128, 64], F32, tag="ctmp")
            nc.vector.tensor_mul(out=ctmp[:], in0=ga[:, 0:64],
                                 in1=ga[:, 128:192])
            if fct2 is not None:
                nc.vector.tensor_add(out=C_blk[:, l, :], in0=ctmp[:],
                                     in1=fct2)
            else:
                nc.vector.tensor_copy(out=C_blk[:, l, :], in_=ctmp[:])
            tct = sm.tile([128, 64], F32, tag="tct")
            nc.scalar.activation(out=tct[:], in_=C_blk[:, l, :], func=AF.Tanh)
            nc.vector.tensor_mul(out=H_blk[:, l, :], in0=ga[:, 64:128],
                                 in1=tct[:])

            if l < T - 1:
                if t < S - 1:
                    h_last(j, l, t, tm, H_blk)
                    # pre-tail for the NEXT step, after the chain-critical
                    # h_last ops so it cannot steal their DVE slot
                    if l + 1 < T - 1:
                        h_pre(j, l + 1, t + 1, tm, H_blk)
                # interleave one deferred far-chunk of prev block's scatter
                if scatter_q:
                    scatter_q.pop(0)()

        # ---- end of block: near chunks (h-side now, c-side next block) ----
        if j < NBLK - 1:
            # boundary tails first: they gate the next block's first steps
            h_tail_boundary(j, 8 * j + T - 1, mk0, H_blk)
            c_pre(T - 1, 8 * j + 8,
                  _ap(mk0[:], [[16, 7], [0, 4], [1, 16]]), G_blk, C_blk)
            ht_build(H_blk)
            ht = state["ht"][1:]
            for (t0, nt, moff) in [(8 * j + 9, 1, 1), (8 * j + 10, 6, 2)]:
                mk_ap = bass.AP(tensor=mk0[:].tensor,
                                offset=mk0[:].offset + moff * (T * BC),
                                ap=mk0[:].ap)
                pe_hside(t0, nt, HOFF[(j, t0)], ht)
                state["pending_cside"].append(
                    (lambda G=G_blk, C=C_blk, Hb=H_blk, m=mk_ap, a=t0, n=nt:
                     scatter_chunk(G, C, Hb, m, a, n, hs_first=False,
                                   c_only=True)))
            # far chunks deferred into next block's steps (lazy mask load)
            for t0 in range(8 * j + 16, S, 8):
                scatter_q.append(
                    (lambda G=G_blk, C=C_blk, Hb=H_blk, jj=j, a=t0, h2=ht:
                     (pe_hside(a, 8, HOFF[(jj, a)], h2),
                      scatter_chunk(G, C, Hb,
                                    load_bulk_mask(jj, a, 8, tag="mkf")[:],
                                    a, 8, hs_first=False))))
            # hs pre-tail for the next block's first step (copy branch)
            h_pre(j + 1, 0, 8 * j + 8, tm, H_blk)
            state["prev_mk0"] = mk0

        state["prev_H"] = H_blk
        state["prev_G"] = G_blk
        state["prev_C"] = C_blk

    while scatter_q:
        scatter_q.pop(0)()

    # ---- output head ----
    prev_H = state["prev_H"]
    opsum = opp.tile([BC, L], F32, tag="o")
    for k1 in range(4):
        nc.tensor.matmul(
            out=opsum[:],
            lhsT=prev_H[:, T - 1, 16 * k1:16 * k1 + 16],
            rhs=wout_s[:, k1, :],
            start=(k1 == 0), stop=(k1 == 3))
    osb = sm.tile([BC, L], F32, tag="osb")
    nc.vector.tensor_add(out=osb[:], in0=opsum[:], in1=bout_s[:])
    nc.gpsimd.dma_start(out=out_d[:], in_=osb[:])


def _host_prep(x, bfs, children, embed, Wix, bix, Wih, bih, Wfx, bfx, Wfh,
               bfh, Wux, bux, Wuh, buh, Wout, bout):
    f32 = np.float32
    wxt = np.zeros((EK, 3 * H), f32)
    for g, (W, bvec) in enumerate([
            (Wix, bix + bih), (Wfx, bfx + bfh), (Wux, bux + buh)]):
        wxt[:E, 512 * g:512 * (g + 1)] = W.T.astype(f32)
        wxt[E, 512 * g:512 * (g + 1)] = bvec.astype(f32)
    wht = np.concatenate(
        [Wih.T, Wfh.T, Wuh.T], axis=1).astype(f32)          # [512, 1536]
    bf16 = mybir.dt.np(mybir.dt.bfloat16)
    wht_hi = wht.astype(bf16)
    wht_lo = (wht - wht_hi.astype(f32)).astype(bf16)
    wout = np.ascontiguousarray(Wout.T.astype(f32))         # [512, L]
    bout_rep = np.tile(bout.astype(f32)[None, :], (BC, 1))  # [16, L]

    xets, wmts, wmks, lmks = [], [], [], []
    for c0 in range(NCORES):
        bs = slice(c0 * BC, (c0 + 1) * BC)
        bfs_c = bfs[bs]
        x_c = x[bs]
        ch_c = children[bs]
        tok = np.take_along_axis(x_c, bfs_c, axis=1)
        xe = embed[tok]
        xet = np.zeros((EK, S * BC), f32)
        xet[:E] = xe.transpose(2, 1, 0).reshape(E, S * BC)
        xet[E] = 1.0
        xets.append(xet)

        wm = np.zeros((S, S, BC), f32)       # [t, tau, b]
        lastw = -np.ones((BC, S), np.int64)
        barange = np.arange(BC)
        for t in range(S):
            cur = bfs_c[:, t]
            ch_of_cur = ch_c[barange, cur, :]
            for b in range(BC):
                nodes = np.nonzero((lastw[b] >= 0) & (ch_of_cur[b] > 0))[0]
                if nodes.size:
                    wm[t, lastw[b, nodes], b] = 1.0
            lastw[barange, cur] = t

        wmt = np.zeros((NBLK * 7, T * BC), f32)
        wmk = np.zeros((WMK_ROWS, T * BC), f32)
        for j in range(NBLK):
            for l in range(1, T):
                wmt[j * 7 + l - 1] = (
                    wm[8 * j + l, 8 * j:8 * j + 8, :].reshape(-1))
            if j < NBLK - 1:
                for r, t in enumerate(range(8 * j + 8, S)):
                    wmk[WMK_OFF[j] + r] = (
                        wm[t, 8 * j:8 * j + 8, :].reshape(-1))
        wmts.append(wmt)
        wmks.append(wmk)

        # block-diagonal L masks for the PE h-side scatter:
        # L[tau*16+b, t_loc*16+b] = wm[t0+t_loc, 8j+tau, b]
        lmk = np.zeros((128, LCOLS), f32)
        for (j, t0, nt) in HCHUNKS:
            off = HOFF[(j, t0)]
            for b in range(BC):
                sub = wm[t0:t0 + nt, 8 * j:8 * j + 8, b]       # [nt, 8]
                rows = np.arange(8) * 16 + b
                cols = off + np.arange(nt) * 16 + b
                lmk[np.ix_(rows, cols)] = sub.T
        lmks.append(lmk.astype(bf16))

    return wxt, wht_hi, wht_lo, wout, bout_rep, xets, wmts, wmks, lmks


def _get_runner(repeat=1):
    rkey = ("runner", repeat)
    if rkey in _prog_cache:
        return _prog_cache[rkey]
    import jax
    from jax.experimental.shard_map import shard_map
    from jax.sharding import Mesh, PartitionSpec
    from concourse import bass2jax

    nc = _build_program(repeat)
    bass2jax.install_neuronx_cc_hook()
    pname = nc.partition_id_tensor.name if nc.partition_id_tensor else None
    in_names, out_names, out_avals, out_shapes, out_dtypes = [], [], [], [], []
    for alloc in nc.m.functions[0].allocations:
        if not isinstance(alloc, mybir.MemoryLocationSet):
            continue
        name = alloc.memorylocations[0].name
        if alloc.kind == "ExternalInput":
            if name != pname:
                in_names.append(name)
        elif alloc.kind == "ExternalOutput":
            out_names.append(name)
            shape = tuple(alloc.tensor_shape)
            dtype = mybir.dt.np(alloc.dtype)
            out_avals.append(jax.core.ShapedArray(shape, dtype))
            out_shapes.append(shape)
            out_dtypes.append(dtype)
    n_params = len(in_names)
    all_in_names = list(in_names) + list(out_names)
    if pname is not None:
        all_in_names.append(pname)
    donate = tuple(range(n_params, n_params + len(out_names)))

    def _body(*args):
        operands = list(args)
        if pname is not None:
            operands.append(bass2jax.partition_id_tensor())
        outs = bass2jax._bass_exec_p.bind(
            *operands,
            out_avals=tuple(out_avals),
            in_names=tuple(all_in_names),
            out_names=tuple(out_names),
            lowering_input_output_aliases=(),
            sim_require_finite=True,
            sim_require_nnan=True,
            nc=nc,
        )
        return tuple(outs)

    devices = jax.devices()[:NCORES]
    mesh = Mesh(np.asarray(devices), ("core",))
    in_specs = (PartitionSpec("core"),) * (n_params + len(out_names))
    out_specs = (PartitionSpec("core"),) * len(out_names)
    sharded = jax.jit(
        shard_map(_body, mesh=mesh, in_specs=in_specs, out_specs=out_specs,
                  check_rep=False),
        donate_argnums=donate, keep_unused=True)
    runner = (sharded, in_names, out_names, out_shapes, out_dtypes)
    _prog_cache[rkey] = runner
    return runner


def _input_data(inputs):
    x = np.asarray(inputs["x"]).astype(np.int64)
    bfs = np.asarray(inputs["bfs"]).astype(np.int64)
    children = np.asarray(inputs["children"]).astype(np.int64)
    embed = np.ascontiguousarray(np.asarray(inputs["embed"], dtype=np.float32))
    wargs = {k: np.asarray(inputs[k], dtype=np.float32)
             for k in ["Wix", "bix", "Wih", "bih", "Wfx", "bfx", "Wfh", "bfh",
                       "Wux", "bux", "Wuh", "buh", "Wout", "bout"]}
    wxt, wht_hi, wht_lo, wout, bout_rep, xets, wmts, wmks, lmks = _host_prep(
        x, bfs, children, embed, **wargs)
    ident = np.eye(128, dtype=np.float32)
    return dict(xet=xets, wxt=[wxt] * NCORES, wht_hi=[wht_hi] * NCORES,
                wht_lo=[wht_lo] * NCORES,
                wout=[wout] * NCORES, bout_rep=[bout_rep] * NCORES,
                wmt=wmts, wmk=wmks, lmk=lmks, ident=[ident] * NCORES)


def kernel(**inputs):
    data = _input_data(inputs)
    sharded, in_names, out_names, out_shapes, out_dtypes = _get_runner()
    concat_in = [np.concatenate(data[nm], axis=0) for nm in in_names]
    zero_outs = [np.zeros((NCORES * sh[0],) + sh[1:], dt)
                 for sh, dt in zip(out_shapes, out_dtypes)]
    try:
        outs = sharded(*concat_in, *zero_outs)
        outs = [np.asarray(o) for o in outs]
    except Exception:
        zero_outs = [np.zeros((NCORES * sh[0],) + sh[1:], dt)
                     for sh, dt in zip(out_shapes, out_dtypes)]
        outs = sharded(*concat_in, *zero_outs)
        outs = [np.asarray(o) for o in outs]
    out = outs[out_names.index("out")]
    return np.ascontiguousarray(out.astype(np.float32))



# revision 86
# speedup vs baseline: 1.0026x; 1.0026x over previous
"""Batch Child-Sum TreeLSTM on 8 NeuronCores — v4 bf16-split + PE-scatter.

v4 over v3 (~1.35ms -> ~1.21ms): (1) the gate GEMM writes f into its OWN
PSUM tile so c_last/gdst/gt-f depend only on the f third of the burst
(whole-tile deps made them wait for the full burst: semaphore increments
land at group stops); (2) h_pre is emitted AFTER h_last so it cannot
steal the DVE slot from the chain-critical hs3 ops (mid-block step went
12.4us with p90 46us at boundaries); (3) block boundaries: the block's
own h-sum into step 8j+8 is computed elementwise from H_blk + mk0 row 0
(off the ht_build/pe_hside chain), and c_pre/c_last extend across the
boundary (prev-block taus 0..6 pre-computed at block end with the mk0
mask, tau 7 via the PSUM G cols at l==0), dropping both 8j+8 scatter
chunks from the critical path.

Data-parallel over batch (16 rows/core).  The O(S^2) masked prefix work is
split into per-block (T=8 steps) scatter passes accumulating fc_acc/hs_acc
for future steps; the per-step chain handles in-block tails (split into an
off-chain pre-tail over taus<l-1 plus a tiny on-chain last term), the gate
GEMM, and the state update.

Gate GEMM (v3): fp32 matmuls serialize LDW+MM at ~517 ns/pair on TRN2
(no FWL, no background weight buffer for fp32).  Instead W and hs2 are
split hi+lo bf16 and three products accumulate in PSUM (W_hi@[h_hi|h_lo]
fused-N + W_lo@h_hi onto the lo cols): 96 bf16 pairs/step ~= 4.0 us vs
24.8 us fp32 (split residual ~2^-17; end-to-end rel err 2.2e-4).

h-side scatter (v3): hs_acc[t] += sum_tau m*H is a per-batch tau-
contraction = PE matmul with a host-built block-diagonal bf16 mask
L[(tau,b),(t,b)], H transposed per block into [tau*16+b, h] bf16 hi/lo
via 4 PE transposes (+c-major staging copy; transpose outputs must land
at PSUM partition 0), and the result transposed back and added into
hs_acc.  Replaces ~700 us of gpsimd/DVE elementwise per run.

Numerics: state C and all accumulators stay fp32 (c reaches ~5e8 and the
recurrence amplifies; plain bf16 on h/C/weights fails the 2e-2 gate).
bf16 hi+lo splits (~17 mantissa bits) are safe everywhere they are used.

Layout: partition p = h % 128; free cols = c*16+b (c = h//128, b =
batch-in-core).  Masks are host-exact, DMA-broadcast via stride-0
partition APs.  GPSIMD cannot touch PSUM (BIR verifier), and DVE reads
at most ONE PSUM operand per instruction (NCC_IBVF027) — hence the
gt = (zx + z_hi) + z_lo ordering.  Engine rates measured in isolation:
Act 0.92, DVE 0.77, GpSimd 0.39 free-els/ns (fp32, AP-shape-insensitive);
in-kernel DVE drops ~2x under multi-engine SBUF contention.
"""

import numpy as np

import concourse.bass as bass
import concourse.bacc as bacc
import concourse.tile as tile
from concourse import mybir

F32 = mybir.dt.float32
BF16 = mybir.dt.bfloat16
AF = mybir.ActivationFunctionType
ALU = mybir.AluOpType

NCORES = 8
B, S, E, H, V, L = 128, 64, 300, 512, 100000, 5
BC = B // NCORES   # 16 batch rows per core
EK = 304           # padded E+bias rows (300 data + 1 ones + 3 zero)
KC = [128, 128, 48]
T = 8              # block size
NBLK = S // T      # 8 blocks
# bulk-mask row offsets per block j (rows of wmk): block j has 56-8j rows
WMK_OFF = [0]
for _j in range(NBLK - 1):
    WMK_OFF.append(WMK_OFF[-1] + (S - 8 * _j - 8))
WMK_ROWS = WMK_OFF[-1] + 8   # 224

LINEARIZE = False
_prog_cache = {}

# h-side scatter chunks (j, t0, nt) in device emission order + L-mask col offs
HCHUNKS = []
for _j in range(NBLK - 1):
    HCHUNKS.append((_j, 8 * _j + 8, 1))
    HCHUNKS.append((_j, 8 * _j + 9, 1))
    HCHUNKS.append((_j, 8 * _j + 10, 6))
    for _t0 in range(8 * _j + 16, S, 8):
        HCHUNKS.append((_j, _t0, 8))
HOFF = {}
_off = 0
for (_j, _t0, _nt) in HCHUNKS:
    HOFF[(_j, _t0)] = _off
    _off += _nt * 16
LCOLS = _off


def _ap(t_ap, dims, doff=0):
    return bass.AP(tensor=t_ap.tensor, offset=t_ap.offset + doff,
                   ap=[t_ap.ap[0], *dims])


def _build_program(repeat=1):
    key = ("nc", repeat)
    if key in _prog_cache:
        return _prog_cache[key]

    nc = bacc.Bacc(None, target_bir_lowering=False, debug=False)

    xet_d = nc.declare_dram_parameter("xet", [EK, S * BC], F32, isOutput=False)
    wxt_d = nc.declare_dram_parameter("wxt", [EK, 3 * H], F32, isOutput=False)
    whi_d = nc.declare_dram_parameter("wht_hi", [H, 3 * H], BF16, isOutput=False)
    wlo_d = nc.declare_dram_parameter("wht_lo", [H, 3 * H], BF16, isOutput=False)
    wout_d = nc.declare_dram_parameter("wout", [H, L], F32, isOutput=False)
    bout_d = nc.declare_dram_parameter("bout_rep", [BC, L], F32, isOutput=False)
    wmt_d = nc.declare_dram_parameter("wmt", [NBLK * 7, T * BC], F32,
                                      isOutput=False)
    wmk_d = nc.declare_dram_parameter("wmk", [WMK_ROWS, T * BC], F32,
                                      isOutput=False)
    lmk_d = nc.declare_dram_parameter("lmk", [128, LCOLS], BF16,
                                      isOutput=False)
    ident_d = nc.declare_dram_parameter("ident", [128, 128], F32,
                                        isOutput=False)
    out_d = nc.declare_dram_parameter("out", [BC, L], F32, isOutput=True)

    with tile.TileContext(nc, linearize=LINEARIZE) as tc:
        with (
            tc.tile_pool(name="singles", bufs=1) as sg,
        ):
            # -------- persistent --------
            zx_all = sg.tile([128, S, 192], F32)   # x-side gates (i,f,u)
            whi_s = sg.tile([128, 4, 3 * H], BF16)
            wlo_s = sg.tile([128, 4, 3 * H], BF16)
            wout_s = sg.tile([128, 4, L], F32)
            bout_s = sg.tile([BC, L], F32)
            fc_acc = sg.tile([128, S, 64], F32)
            hs_acc = sg.tile([128, S, 64], F32)
            ident_s = sg.tile([128, 128], F32)
            nc.gpsimd.dma_start(out=ident_s[:], in_=ident_d[:])

            nc.gpsimd.dma_start(
                out=whi_s[:],
                in_=whi_d[:].rearrange("(k1 p) n -> p k1 n", p=128))
            nc.gpsimd.dma_start(
                out=wlo_s[:],
                in_=wlo_d[:].rearrange("(k1 p) n -> p k1 n", p=128))
            nc.gpsimd.dma_start(
                out=wout_s[:],
                in_=wout_d[:].rearrange("(k1 p) n -> p k1 n", p=128))
            nc.gpsimd.dma_start(out=bout_s[:], in_=bout_d[:])

            # -------- phase 1: x-side GEMM --------
            with (
                tc.tile_pool(name="xw", bufs=1) as xw,
                tc.tile_pool(name="ph1p", bufs=2, space="PSUM") as ph1p,
            ):
                wxt_c = [xw.tile([128, 3 * H], F32, name=f"wxt{k}",
                                 tag=f"wxt{k}") for k in range(3)]
                xet_c = [xw.tile([128, S * BC], F32, name=f"xet{k}",
                                 tag=f"xet{k}") for k in range(3)]
                for k1 in range(3):
                    cnt = KC[k1]
                    nc.gpsimd.dma_start(out=wxt_c[k1][0:cnt, :],
                                        in_=wxt_d[k1 * 128:k1 * 128 + cnt, :])
                    nc.gpsimd.dma_start(out=xet_c[k1][0:cnt, :],
                                        in_=xet_d[k1 * 128:k1 * 128 + cnt, :])
                for nh in range(2):
                    for g in range(3):
                        for c in range(4):
                            zxp = ph1p.tile([128, 512], F32, tag="zxp")
                            for k1 in range(3):
                                cnt = KC[k1]
                                nc.tensor.matmul(
                                    out=zxp[:],
                                    lhsT=wxt_c[k1][0:cnt,
                                                   512 * g + 128 * c:
                                                   512 * g + 128 * c + 128],
                                    rhs=xet_c[k1][0:cnt,
                                                  512 * nh:512 * nh + 512],
                                    start=(k1 == 0), stop=(k1 == 2))
                            # psum cols (t, b) -> zx_all[:, t, g*64+c*16+b]
                            dst = _ap(zx_all[:], [[192, 32], [1, 16]],
                                      doff=(nh * 32) * 192 + g * 64 + c * 16)
                            nc.scalar.activation(
                                out=dst,
                                in_=zxp[:].rearrange("p (t b) -> p t b", b=16),
                                func=AF.Copy)

            # -------- phase 2 --------
            with (
                tc.tile_pool(name="st", bufs=2) as stp,     # block state
                tc.tile_pool(name="sm", bufs=3) as sm,      # small per-step
                tc.tile_pool(name="mk", bufs=2) as mkp,     # bulk masks
                tc.tile_pool(name="mt", bufs=2) as mtp,     # tail masks
                tc.tile_pool(name="sp", bufs=2) as spp,     # scatter scratch
                tc.tile_pool(name="hts", bufs=2) as hts,    # H^T bf16 hi/lo
                tc.tile_pool(name="lmp", bufs=2) as lmp,    # L masks
                tc.tile_pool(name="zsp", bufs=1) as zsp,    # hside sbuf stage
                tc.tile_pool(name="zp", bufs=2, space="PSUM") as zp,
                tc.tile_pool(name="htp", bufs=1, space="PSUM") as htp,
                tc.tile_pool(name="sc", bufs=2, space="PSUM") as scp,
                tc.tile_pool(name="op", bufs=1, space="PSUM") as opp,
            ):
                env = dict(locals())
                for _rep in range(repeat):
                    _phase2(nc, env)

    nc.finalize()
    _prog_cache[key] = nc
    return nc


def _phase2(nc, env):
    zx_all = env["zx_all"]; whi_s = env["whi_s"]; wlo_s = env["wlo_s"]
    wout_s = env["wout_s"]
    bout_s = env["bout_s"]; fc_acc = env["fc_acc"]; hs_acc = env["hs_acc"]
    wmt_d = env["wmt_d"]; wmk_d = env["wmk_d"]; out_d = env["out_d"]
    lmk_d = env["lmk_d"]; ident_s = env["ident_s"]
    stp = env["stp"]; sm = env["sm"]; mkp = env["mkp"]; mtp = env["mtp"]
    spp = env["spp"]; zp = env["zp"]; opp = env["opp"]
    hts = env["hts"]; lmp = env["lmp"]; zsp = env["zsp"]
    htp = env["htp"]; scp = env["scp"]; tc = env["tc"]

    nc.vector.memset(fc_acc[:], 0.0)
    nc.gpsimd.memset(hs_acc[:], 0.0)

    state = {"hs3": None, "prev_H": None, "fct2": None, "fct2_pre": None,
             "hs_pre": None, "prev_G": None, "pending_cside": []}
    scatter_q = []  # deferred far-chunk thunks of the previous block

    def load_tail_mask(j):
        tm = mtp.tile([128, 7, T * BC], F32, tag="tm")
        src = bass.AP(tensor=wmt_d[:].tensor,
                      offset=wmt_d[:].offset + (j * 7) * (T * BC),
                      ap=[[0, 128], [T * BC, 7], [1, T * BC]])
        nc.gpsimd.dma_start(out=tm[:], in_=src)
        return tm

    def load_bulk_mask(j, t0, nt, tag):
        mk = mkp.tile([128, 8, T * BC], F32, tag=tag)
        row = WMK_OFF[j] + (t0 - 8 * j - 8)
        src = bass.AP(tensor=wmk_d[:].tensor,
                      offset=wmk_d[:].offset + row * (T * BC),
                      ap=[[0, 128], [T * BC, nt], [1, T * BC]])
        nc.gpsimd.dma_start(
            out=_ap(mk[:], [[T * BC, nt], [1, T * BC]]), in_=src)
        return mk

    def ht_build(H_blk):
        """Transpose the block's H into H^T bf16 hi/lo tiles [tau*16+b, h]."""
        ht_ps, ht_hi, ht_lo = state["ht"]
        # stage c-major so each per-c transpose input is contiguous
        st_h = zsp.tile([128, 4, 128], F32, tag="sth")
        nc.gpsimd.tensor_copy(
            out=_ap(st_h[:], [[16, 8], [128, 4], [1, 16]]),
            in_=_ap(H_blk[:], [[64, 8], [16, 4], [1, 16]]))
        for c in range(4):
            nc.tensor.transpose(
                out=ht_ps[:, c * 128:c * 128 + 128],
                in_=st_h[:, c, :], identity=ident_s[:])
            nc.scalar.activation(
                out=ht_hi[:, c, :],
                in_=ht_ps[:, c * 128:c * 128 + 128], func=AF.Copy)
            nc.vector.tensor_sub(
                out=ht_lo[:, c, :],
                in0=ht_ps[:, c * 128:c * 128 + 128],
                in1=ht_hi[:, c, :])

    def pe_hside(t0, nt, off, ht, prio=0):
        """hs_acc[t0:t0+nt] += masked sum of block H via PE matmul."""
        if prio:
            with tc.high_priority(offset=prio):
                return _pe_hside(t0, nt, off, ht)
        return _pe_hside(t0, nt, off, ht)

    def _pe_hside(t0, nt, off, ht):
        ht_hi, ht_lo = ht
        m = nt * 16
        lt = lmp.tile([128, 128], BF16, tag="lm")
        nc.gpsimd.dma_start(out=lt[:, 0:m], in_=lmk_d[:, off:off + m])
        zT = scp.tile([128, 512], F32, tag="sc")
        for c in range(4):
            nc.tensor.matmul(out=zT[0:m, c * 128:c * 128 + 128],
                             lhsT=lt[:, 0:m], rhs=ht_hi[:, c, :],
                             start=True, stop=False, skip_group_check=True)
            nc.tensor.matmul(out=zT[0:m, c * 128:c * 128 + 128],
                             lhsT=lt[:, 0:m], rhs=ht_lo[:, c, :],
                             start=False, stop=True, skip_group_check=True)
        zs = zsp.tile([128, 512], F32, tag="zs")
        nc.scalar.activation(out=zs[0:m, :], in_=zT[0:m, :], func=AF.Copy)
        tb = scp.tile([128, 512], F32, tag="sc")
        for c in range(4):
            nc.tensor.transpose(out=tb[:, c * m:c * m + m],
                                in_=zs[0:m, c * 128:c * 128 + 128],
                                identity=ident_s[0:m, 0:m])
            nc.vector.tensor_add(
                out=_ap(hs_acc[:], [[64, nt], [1, 16]],
                        doff=t0 * 64 + c * 16),
                in0=_ap(hs_acc[:], [[64, nt], [1, 16]],
                        doff=t0 * 64 + c * 16),
                in1=_ap(tb[:], [[16, nt], [1, 16]], doff=c * m))

    def scatter_chunk(G_blk, C_blk, H_blk, mk_ap, t0, nt, hs_first,
                      h_only=False, c_only=False):
        _scatter_chunk(G_blk, C_blk, H_blk, mk_ap, t0, nt)

    def _scatter_chunk(G_blk, C_blk, H_blk, mk_ap, t0, nt):
        """Contributions of block cols to t in [t0, t0+nt)."""
        s1 = spp.tile([128, T, 8 * 64], F32, tag="s1")
        m_ap = bass.AP(tensor=mk_ap.tensor, offset=mk_ap.offset,
                       ap=[mk_ap.ap[0],
                           [T * BC, nt], [16, 8], [0, 4], [1, 16]])
        d3 = [[512, nt], [64, 8], [1, 64]]
        d4 = [[512, nt], [64, 8], [16, 4], [1, 16]]

        def cside():
            nc.vector.tensor_add(
                out=_ap(s1[:], d3),
                in0=_ap(G_blk[:], [[0, nt], [64, 8], [1, 64]]),
                in1=_ap(zx_all[:], [[192, nt], [0, 8], [1, 64]],
                        doff=t0 * 192 + 64))
            nc.scalar.activation(out=s1[0:128, 0:nt, :],
                                 in_=s1[0:128, 0:nt, :], func=AF.Sigmoid)
            nc.vector.tensor_mul(
                out=_ap(s1[:], d3),
                in0=_ap(s1[:], d3),
                in1=_ap(C_blk[:], [[0, nt], [64, 8], [1, 64]]))
            nc.gpsimd.tensor_mul(
                out=_ap(s1[:], d4), in0=_ap(s1[:], d4), in1=m_ap)
            nc.vector.tensor_add(
                out=_ap(s1[:], [[512, nt], [64, 4], [1, 64]]),
                in0=_ap(s1[:], [[512, nt], [64, 4], [1, 64]]),
                in1=_ap(s1[:], [[512, nt], [64, 4], [1, 64]], doff=256))
            nc.vector.tensor_add(
                out=_ap(s1[:], [[512, nt], [64, 2], [1, 64]]),
                in0=_ap(s1[:], [[512, nt], [64, 2], [1, 64]]),
                in1=_ap(s1[:], [[512, nt], [64, 2], [1, 64]], doff=128))
            nc.gpsimd.tensor_add(
                out=_ap(s1[:], [[512, nt], [1, 64]]),
                in0=_ap(s1[:], [[512, nt], [1, 64]]),
                in1=_ap(s1[:], [[512, nt], [1, 64]], doff=64))
            nc.gpsimd.tensor_add(
                out=_ap(fc_acc[:], [[64, nt], [1, 64]], doff=t0 * 64),
                in0=_ap(fc_acc[:], [[64, nt], [1, 64]], doff=t0 * 64),
                in1=_ap(s1[:], [[512, nt], [1, 64]]))

        cside()

    def h_tail_boundary(j, t, mk0, H_blk):
        """hs2 for step t+1 = hs_acc[t+1] (far) + full-block masked sum.

        Keeps the block-boundary off the ht_build/pe_hside chain: the
        block's own contribution to step 8j+8 is computed elementwise
        from H_blk with mk0 row 0 instead of a PE scatter chunk.
        """
        hs2n = sm.tile([128, 4, 32], F32, tag="hs2")
        st2 = sm.tile([128, T, 64], F32, tag="st2")
        nc.gpsimd.tensor_mul(
            out=_ap(st2[:], [[64, 8], [16, 4], [1, 16]]),
            in0=_ap(H_blk[:], [[64, 8], [16, 4], [1, 16]]),
            in1=_ap(mk0[:], [[16, 8], [0, 4], [1, 16]]))
        hsr = sm.tile([128, 64], F32, tag="hsr")
        nc.vector.tensor_reduce(
            out=hsr[:], in_=_ap(st2[:], [[1, 64], [64, 8]]),
            axis=mybir.AxisListType.X, op=ALU.add)
        nc.gpsimd.tensor_add(
            out=_ap(hs2n[:], [[32, 4], [1, 16]]),
            in0=hsr[:].rearrange("p (c b) -> p c b", b=16),
            in1=hs_acc[:, t + 1, :].rearrange("p (c b) -> p c b", b=16))
        nc.gpsimd.tensor_copy(
            out=_ap(hs2n[:], [[32, 4], [1, 16]], doff=16),
            in_=H_blk[:, T - 1, :].rearrange("p (c b) -> p c b", b=16))
        hs3n = sm.tile([128, 4, 64], BF16, tag="hs3")
        nc.scalar.activation(out=hs3n[:, :, 0:32], in_=hs2n[:], func=AF.Copy)
        nc.vector.tensor_sub(out=hs3n[:, :, 32:64], in0=hs2n[:],
                             in1=hs3n[:, :, 0:32])
        state["hs3"] = hs3n

    def h_pre(j, l, t, tm, H_blk):
        """h_sum pre-tail for step t+1: taus 0..l-1 plus hs_acc[t+1]."""
        hsp = sm.tile([128, 4, 16], F32, tag="hsp")
        if l > 0:
            st2 = sm.tile([128, T, 64], F32, tag="st2")
            nc.gpsimd.tensor_mul(
                out=_ap(st2[:], [[64, l], [16, 4], [1, 16]]),
                in0=_ap(H_blk[:], [[64, l], [16, 4], [1, 16]]),
                in1=_ap(tm[:], [[16, l], [0, 4], [1, 16]],
                        doff=l * (T * BC)))
            hsr = sm.tile([128, 64], F32, tag="hsr")
            nc.vector.tensor_reduce(
                out=hsr[:], in_=_ap(st2[:], [[1, 64], [64, l]]),
                axis=mybir.AxisListType.X, op=ALU.add)
            nc.gpsimd.tensor_add(
                out=_ap(hsp[:], [[16, 4], [1, 16]]),
                in0=hsr[:].rearrange("p (c b) -> p c b", b=16),
                in1=hs_acc[:, t + 1, :].rearrange("p (c b) -> p c b", b=16))
        else:
            nc.gpsimd.tensor_copy(
                out=_ap(hsp[:], [[16, 4], [1, 16]]),
                in_=hs_acc[:, t + 1, :].rearrange("p (c b) -> p c b", b=16))
        state["hs_pre"] = hsp

    def h_last(j, l, t, tm, H_blk):
        """Finish hs2/hs3 for step t+1 with the tau=l last term."""
        hs2n = sm.tile([128, 4, 32], F32, tag="hs2")
        hlm = sm.tile([128, 64], F32, tag="hlm")
        nc.gpsimd.tensor_mul(
            out=_ap(hlm[:], [[16, 4], [1, 16]]),
            in0=H_blk[:, l, :].rearrange("p (c b) -> p c b", b=16),
            in1=_ap(tm[:], [[0, 4], [1, 16]],
                    doff=l * (T * BC) + l * 16))
        nc.vector.tensor_add(
            out=_ap(hs2n[:], [[32, 4], [1, 16]]),
            in0=_ap(state["hs_pre"][:], [[16, 4], [1, 16]]),
            in1=hlm[:].rearrange("p (c b) -> p c b", b=16))
        nc.gpsimd.tensor_copy(
            out=_ap(hs2n[:], [[32, 4], [1, 16]], doff=16),
            in_=H_blk[:, l, :].rearrange("p (c b) -> p c b", b=16))
        hs3n = sm.tile([128, 4, 64], BF16, tag="hs3")
        nc.scalar.activation(out=hs3n[:, :, 0:32], in_=hs2n[:], func=AF.Copy)
        nc.vector.tensor_sub(out=hs3n[:, :, 32:64], in0=hs2n[:],
                             in1=hs3n[:, :, 0:32])
        state["hs3"] = hs3n

    def c_pre(nl, tnext, mask_ap, G_blk, C_blk):
        """C-side pre-tail for step tnext: taus 0..nl-1 plus fc_acc."""
        st = sm.tile([128, T, 64], F32, tag="st")
        nc.vector.tensor_add(
            out=_ap(st[:], [[64, nl], [1, 64]]),
            in0=G_blk[:, 0:nl, :],
            in1=_ap(zx_all[:], [[0, nl], [1, 64]],
                    doff=tnext * 192 + 64))
        nc.scalar.activation(out=st[:, 0:nl, :], in_=st[:, 0:nl, :],
                             func=AF.Sigmoid)
        nc.vector.tensor_mul(out=st[:, 0:nl, :], in0=st[:, 0:nl, :],
                             in1=C_blk[:, 0:nl, :])
        nc.vector.tensor_mul(
            out=_ap(st[:], [[64, nl], [16, 4], [1, 16]]),
            in0=_ap(st[:], [[64, nl], [16, 4], [1, 16]]),
            in1=mask_ap)
        fct = sm.tile([128, 64], F32, tag="fct")
        nc.vector.tensor_reduce(
            out=fct[:], in_=_ap(st[:], [[1, 64], [64, nl]]),
            axis=mybir.AxisListType.X, op=ALU.add)
        fct2p = sm.tile([128, 64], F32, tag="fct2p")
        nc.vector.tensor_add(out=fct2p[:], in0=fct[:],
                             in1=fc_acc[:, tnext, :])
        state["fct2_pre"] = fct2p

    def c_last(t, zpsF, C_col, mask_ap):
        """fct2 for step t = pre-tail + last term (fresh G from PSUM)."""
        stl = sm.tile([128, 64], F32, tag="stl")
        nc.vector.tensor_add(
            out=stl[:], in0=zx_all[:, t, 64:128],
            in1=_ap(zpsF[:], [[64, 4], [1, 16]], doff=16))
        nc.vector.tensor_add(
            out=stl[:], in0=stl[:],
            in1=_ap(zpsF[:], [[64, 4], [1, 16]], doff=48))
        nc.scalar.activation(out=stl[:], in_=stl[:], func=AF.Sigmoid)
        nc.vector.tensor_mul(out=stl[:], in0=stl[:], in1=C_col)
        nc.gpsimd.tensor_mul(
            out=_ap(stl[:], [[16, 4], [1, 16]]),
            in0=_ap(stl[:], [[16, 4], [1, 16]]),
            in1=mask_ap)
        fct2 = sm.tile([128, 64], F32, tag="fct2")
        pre = state["fct2_pre"]
        nc.vector.tensor_add(
            out=fct2[:], in0=stl[:],
            in1=(fc_acc[:, t, :] if pre is None else pre[:]))
        state["fct2"] = fct2[:]

    for j in range(NBLK):
        G_blk = stp.tile([128, T, 64], F32, tag="G")
        C_blk = stp.tile([128, T, 64], F32, tag="C")
        H_blk = stp.tile([128, T, 64], F32, tag="H")
        tm = load_tail_mask(j)
        mk0 = (load_bulk_mask(j, 8 * j + 8, 8, tag="mk0")
               if j < NBLK - 1 else None)
        if j < NBLK - 1:
            ht_ps = htp.tile([128, 512], F32, tag="htp", name="ht_ps")
            ht_hi = hts.tile([128, 4, 128], BF16, tag="hthi", name="ht_hi")
            ht_lo = hts.tile([128, 4, 128], BF16, tag="htlo", name="ht_lo")
            state["ht"] = (ht_ps, ht_hi, ht_lo)

        for l in range(T):
            t = 8 * j + l
            zxs = zx_all[:, t, :]
            ga = sm.tile([128, 192], F32, tag="ga")
            if l == 0 and j == 0:
                h_pre(j, 0, 0, tm, H_blk)
            if t == 0:
                gt = sm.tile([128, 192], F32, tag="gt")
                nc.gpsimd.tensor_copy(out=gt[:], in_=zxs)
            else:
                hs3 = state["hs3"]
                # bf16 3-pass split: W_hi@[h_hi|h_lo] fused-N + W_lo@h_hi
                # (accumulated onto the lo cols). f gets its OWN psum tile
                # so c_last/gdst/gt-f wait only on the f MMs (first third
                # of the burst), not the whole-tile dependency.
                zpsF = zp.tile([128, 256], F32, tag="zf")
                zpsIU = zp.tile([128, 256], F32, tag="zu")
                for g, wof, ztile, base, nn in (
                        (1, 512, zpsF, 0, 32), (0, 0, zpsIU, 0, 16),
                        (2, 1024, zpsIU, 128, 16)):
                    for c in range(4):
                        o0 = base + 2 * nn * c
                        for k1 in range(4):
                            if nn == 16:
                                rfused = _ap(hs3[:, k1, :], [[32, 2], [1, 16]])
                                rhi = hs3[:, k1, 0:16]
                            else:
                                rfused = hs3[:, k1, 0:64]
                                rhi = hs3[:, k1, 0:32]
                            nc.tensor.matmul(
                                out=ztile[:, o0:o0 + 2 * nn],
                                lhsT=whi_s[:, k1,
                                           wof + 128 * c:wof + 128 * c + 128],
                                rhs=rfused,
                                start=(k1 == 0), stop=False,
                                skip_group_check=True)
                            nc.tensor.matmul(
                                out=ztile[:, o0 + nn:o0 + 2 * nn],
                                lhsT=wlo_s[:, k1,
                                           wof + 128 * c:wof + 128 * c + 128],
                                rhs=rhi,
                                start=False, stop=(k1 == 3),
                                skip_group_check=True)
                # fct2 for this step straight from the f-gate PSUM cols
                if l > 0:
                    c_last(t, zpsF, C_blk[:, l - 1, :],
                           _ap(tm[:], [[0, 4], [1, 16]],
                               doff=(l - 1) * (T * BC) + (l - 1) * 16))
                else:
                    # boundary: last term is tau=7 of the previous block
                    c_last(t, zpsF, state["prev_C"][:, T - 1, :],
                           _ap(state["prev_mk0"][:], [[0, 4], [1, 16]],
                               doff=(T - 1) * 16))
                    state["fct2_pre"] = None
                # G col of step t-1 = hp products (hi cols + lo cols);
                # feeds only c_pre/scatter, off the serial chain
                gdst = (G_blk[:, l - 1, :] if l > 0
                        else state["prev_G"][:, T - 1, :])
                nc.scalar.activation(
                    out=gdst,
                    in_=_ap(zpsF[:], [[64, 4], [1, 16]], doff=16),
                    func=AF.Copy)
                nc.vector.tensor_add(
                    out=gdst, in0=gdst,
                    in1=_ap(zpsF[:], [[64, 4], [1, 16]], doff=48))
                if l == 0:
                    for th in state["pending_cside"]:
                        th()
                    state["pending_cside"] = []
                # gates: gt = (z_hi + zx) + z_lo  (i | f | u); one PSUM
                # operand per DVE instruction (walrus NCC_IBVF027)
                gt = sm.tile([128, 192], F32, tag="gt")
                nc.vector.tensor_add(
                    out=gt[:, 64:128], in0=zxs[:, 64:128],
                    in1=_ap(zpsF[:], [[64, 4], [1, 16]], doff=0))
                nc.vector.tensor_add(
                    out=gt[:, 64:128], in0=gt[:, 64:128],
                    in1=_ap(zpsF[:], [[64, 4], [1, 16]], doff=32))
                nc.vector.tensor_add(
                    out=gt[:, 0:64], in0=zxs[:, 0:64],
                    in1=_ap(zpsIU[:], [[32, 4], [1, 16]], doff=0))
                nc.vector.tensor_add(
                    out=gt[:, 128:192], in0=zxs[:, 128:192],
                    in1=_ap(zpsIU[:], [[32, 4], [1, 16]], doff=128))
                nc.vector.tensor_add(
                    out=gt[:, 0:64], in0=gt[:, 0:64],
                    in1=_ap(zpsIU[:], [[32, 4], [1, 16]], doff=16))
                nc.vector.tensor_add(
                    out=gt[:, 128:192], in0=gt[:, 128:192],
                    in1=_ap(zpsIU[:], [[32, 4], [1, 16]], doff=128 + 16))
                # pre-tail for step t+1 (off-chain; uses fresh G[l-1])
                if 0 < l < T - 1 and t < S - 1:
                    c_pre(l, t + 1,
                          _ap(tm[:], [[16, l], [0, 4], [1, 16]],
                              doff=l * (T * BC)),
                          G_blk, C_blk)
            nc.scalar.activation(out=ga[:, 0:128], in_=gt[:, 0:128],
                                 func=AF.Sigmoid)
            nc.scalar.activation(out=ga[:, 128:192], in_=gt[:, 128:192],
                                 func=AF.Tanh)

            # ---- c, h ----  (fct2 = pre-tail + last term, set by c_last)
            fct2 = state["fct2"] if t > 0 else None
            ctmp = sm.tile([128, 64], F32, tag="ctmp")
            nc.vector.tensor_mul(out=ctmp[:], in0=ga[:, 0:64],
                                 in1=ga[:, 128:192])
            if fct2 is not None:
                nc.vector.tensor_add(out=C_blk[:, l, :], in0=ctmp[:],
                                     in1=fct2)
            else:
                nc.vector.tensor_copy(out=C_blk[:, l, :], in_=ctmp[:])
            tct = sm.tile([128, 64], F32, tag="tct")
            nc.scalar.activation(out=tct[:], in_=C_blk[:, l, :], func=AF.Tanh)
            nc.vector.tensor_mul(out=H_blk[:, l, :], in0=ga[:, 64:128],
                                 in1=tct[:])

            if l < T - 1:
                if t < S - 1:
                    h_last(j, l, t, tm, H_blk)
                    # pre-tail for the NEXT step, after the chain-critical
                    # h_last ops so it cannot steal their DVE slot
                    if l + 1 < T - 1:
                        h_pre(j, l + 1, t + 1, tm, H_blk)
                # interleave one deferred far-chunk of prev block's scatter
                if scatter_q:
                    scatter_q.pop(0)()

        # ---- end of block: near chunks (h-side now, c-side next block) ----
        if j < NBLK - 1:
            # boundary tails first: they gate the next block's first steps
            h_tail_boundary(j, 8 * j + T - 1, mk0, H_blk)
            c_pre(T - 1, 8 * j + 8,
                  _ap(mk0[:], [[16, 7], [0, 4], [1, 16]]), G_blk, C_blk)
            ht_build(H_blk)
            ht = state["ht"][1:]
            for (t0, nt, moff) in [(8 * j + 9, 1, 1), (8 * j + 10, 6, 2)]:
                mk_ap = bass.AP(tensor=mk0[:].tensor,
                                offset=mk0[:].offset + moff * (T * BC),
                                ap=mk0[:].ap)
                pe_hside(t0, nt, HOFF[(j, t0)], ht)
                state["pending_cside"].append(
                    (lambda G=G_blk, C=C_blk, Hb=H_blk, m=mk_ap, a=t0, n=nt:
                     scatter_chunk(G, C, Hb, m, a, n, hs_first=False,
                                   c_only=True)))
            # far chunks deferred into next block's steps (lazy mask load)
            for t0 in range(8 * j + 16, S, 8):
                scatter_q.append(
                    (lambda G=G_blk, C=C_blk, Hb=H_blk, jj=j, a=t0, h2=ht:
                     (pe_hside(a, 8, HOFF[(jj, a)], h2),
                      scatter_chunk(G, C, Hb,
                                    load_bulk_mask(jj, a, 8, tag="mkf")[:],
                                    a, 8, hs_first=False))))
            # hs pre-tail for the next block's first step (copy branch)
            h_pre(j + 1, 0, 8 * j + 8, tm, H_blk)
            state["prev_mk0"] = mk0

        state["prev_H"] = H_blk
        state["prev_G"] = G_blk
        state["prev_C"] = C_blk

    while scatter_q:
        scatter_q.pop(0)()

    # ---- output head ----
    prev_H = state["prev_H"]
    opsum = opp.tile([BC, L], F32, tag="o")
    for k1 in range(4):
        nc.tensor.matmul(
            out=opsum[:],
            lhsT=prev_H[:, T - 1, 16 * k1:16 * k1 + 16],
            rhs=wout_s[:, k1, :],
            start=(k1 == 0), stop=(k1 == 3))
    osb = sm.tile([BC, L], F32, tag="osb")
    nc.vector.tensor_add(out=osb[:], in0=opsum[:], in1=bout_s[:])
    nc.gpsimd.dma_start(out=out_d[:], in_=osb[:])


def _host_prep(x, bfs, children, embed, Wix, bix, Wih, bih, Wfx, bfx, Wfh,
               bfh, Wux, bux, Wuh, buh, Wout, bout):
    f32 = np.float32
    wxt = np.zeros((EK, 3 * H), f32)
    for g, (W, bvec) in enumerate([
            (Wix, bix + bih), (Wfx, bfx + bfh), (Wux, bux + buh)]):
        wxt[:E, 512 * g:512 * (g + 1)] = W.T.astype(f32)
        wxt[E, 512 * g:512 * (g + 1)] = bvec.astype(f32)
    wht = np.concatenate(
        [Wih.T, Wfh.T, Wuh.T], axis=1).astype(f32)          # [512, 1536]
    bf16 = mybir.dt.np(mybir.dt.bfloat16)
    wht_hi = wht.astype(bf16)
    wht_lo = (wht - wht_hi.astype(f32)).astype(bf16)
    wout = np.ascontiguousarray(Wout.T.astype(f32))         # [512, L]
    bout_rep = np.tile(bout.astype(f32)[None, :], (BC, 1))  # [16, L]

    xets, wmts, wmks, lmks = [], [], [], []
    for c0 in range(NCORES):
        bs = slice(c0 * BC, (c0 + 1) * BC)
        bfs_c = bfs[bs]
        x_c = x[bs]
        ch_c = children[bs]
        tok = np.take_along_axis(x_c, bfs_c, axis=1)
        xe = embed[tok]
        xet = np.zeros((EK, S * BC), f32)
        xet[:E] = xe.transpose(2, 1, 0).reshape(E, S * BC)
        xet[E] = 1.0
        xets.append(xet)

        wm = np.zeros((S, S, BC), f32)       # [t, tau, b]
        lastw = -np.ones((BC, S), np.int64)
        barange = np.arange(BC)
        for t in range(S):
            cur = bfs_c[:, t]
            ch_of_cur = ch_c[barange, cur, :]
            for b in range(BC):
                nodes = np.nonzero((lastw[b] >= 0) & (ch_of_cur[b] > 0))[0]
                if nodes.size:
                    wm[t, lastw[b, nodes], b] = 1.0
            lastw[barange, cur] = t

        wmt = np.zeros((NBLK * 7, T * BC), f32)
        wmk = np.zeros((WMK_ROWS, T * BC), f32)
        for j in range(NBLK):
            for l in range(1, T):
                wmt[j * 7 + l - 1] = (
                    wm[8 * j + l, 8 * j:8 * j + 8, :].reshape(-1))
            if j < NBLK - 1:
                for r, t in enumerate(range(8 * j + 8, S)):
                    wmk[WMK_OFF[j] + r] = (
                        wm[t, 8 * j:8 * j + 8, :].reshape(-1))
        wmts.append(wmt)
        wmks.append(wmk)

        # block-diagonal L masks for the PE h-side scatter:
        # L[tau*16+b, t_loc*16+b] = wm[t0+t_loc, 8j+tau, b]
        lmk = np.zeros((128, LCOLS), f32)
        for (j, t0, nt) in HCHUNKS:
            off = HOFF[(j, t0)]
            for b in range(BC):
                sub = wm[t0:t0 + nt, 8 * j:8 * j + 8, b]       # [nt, 8]
                rows = np.arange(8) * 16 + b
                cols = off + np.arange(nt) * 16 + b
                lmk[np.ix_(rows, cols)] = sub.T
        lmks.append(lmk.astype(bf16))

    return wxt, wht_hi, wht_lo, wout, bout_rep, xets, wmts, wmks, lmks


def _get_runner(repeat=1):
    rkey = ("runner", repeat)
    if rkey in _prog_cache:
        return _prog_cache[rkey]
    import jax
    from jax.experimental.shard_map import shard_map
    from jax.sharding import Mesh, PartitionSpec
    from concourse import bass2jax

    nc = _build_program(repeat)
    bass2jax.install_neuronx_cc_hook()
    pname = nc.partition_id_tensor.name if nc.partition_id_tensor else None
    in_names, out_names, out_avals, out_shapes, out_dtypes = [], [], [], [], []
    for alloc in nc.m.functions[0].allocations:
        if not isinstance(alloc, mybir.MemoryLocationSet):
            continue
        name = alloc.memorylocations[0].name
        if alloc.kind == "ExternalInput":
            if name != pname:
                in_names.append(name)
        elif alloc.kind == "ExternalOutput":
            out_names.append(name)
            shape = tuple(alloc.tensor_shape)
            dtype = mybir.dt.np(alloc.dtype)
            out_avals.append(jax.core.ShapedArray(shape, dtype))
            out_shapes.append(shape)
            out_dtypes.append(dtype)
    n_params = len(in_names)
    all_in_names = list(in_names) + list(out_names)
    if pname is not None:
        all_in_names.append(pname)
    donate = tuple(range(n_params, n_params + len(out_names)))

    def _body(*args):
        operands = list(args)
        if pname is not None:
            operands.append(bass2jax.partition_id_tensor())
        outs = bass2jax._bass_exec_p.bind(
            *operands,
            out_avals=tuple(out_avals),
            in_names=tuple(all_in_names),
            out_names=tuple(out_names),
            lowering_input_output_aliases=(),
            sim_require_finite=True,
            sim_require_nnan=True,
            nc=nc,
        )
        return tuple(outs)

    devices = jax.devices()[:NCORES]
    mesh = Mesh(np.asarray(devices), ("core",))
    in_specs = (PartitionSpec("core"),) * (n_params + len(out_names))
    out_specs = (PartitionSpec("core"),) * len(out_names)
    sharded = jax.jit(
        shard_map(_body, mesh=mesh, in_specs=in_specs, out_specs=out_specs,
                  check_rep=False),
        donate_argnums=donate, keep_unused=True)
    runner = (sharded, in_names, out_names, out_shapes, out_dtypes)
    _prog_cache[rkey] = runner
    return runner


def _input_data(inputs):
    x = np.asarray(inputs["x"]).astype(np.int64)
    bfs = np.asarray(inputs["bfs"]).astype(np.int64)
    children = np.asarray(inputs["children"]).astype(np.int64)
    embed = np.ascontiguousarray(np.asarray(inputs["embed"], dtype=np.float32))
    wargs = {k: np.asarray(inputs[k], dtype=np.float32)
             for k in ["Wix", "bix", "Wih", "bih", "Wfx", "bfx", "Wfh", "bfh",
                       "Wux", "bux", "Wuh", "buh", "Wout", "bout"]}
    wxt, wht_hi, wht_lo, wout, bout_rep, xets, wmts, wmks, lmks = _host_prep(
        x, bfs, children, embed, **wargs)
    ident = np.eye(128, dtype=np.float32)
    return dict(xet=xets, wxt=[wxt] * NCORES, wht_hi=[wht_hi] * NCORES,
                wht_lo=[wht_lo] * NCORES,
                wout=[wout] * NCORES, bout_rep=[bout_rep] * NCORES,
                wmt=wmts, wmk=wmks, lmk=lmks, ident=[ident] * NCORES)


def kernel(**inputs):
    data = _input_data(inputs)
    sharded, in_names, out_names, out_shapes, out_dtypes = _get_runner()
    concat_in = [np.concatenate(data[nm], axis=0) for nm in in_names]
    zero_outs = [np.zeros((NCORES * sh[0],) + sh[1:], dt)
                 for sh, dt in zip(out_shapes, out_dtypes)]
    try:
        outs = sharded(*concat_in, *zero_outs)
        outs = [np.asarray(o) for o in outs]
    except Exception:
        zero_outs = [np.zeros((NCORES * sh[0],) + sh[1:], dt)
                     for sh, dt in zip(out_shapes, out_dtypes)]
        outs = sharded(*concat_in, *zero_outs)
        outs = [np.asarray(o) for o in outs]
    out = outs[out_names.index("out")]
    return np.ascontiguousarray(out.astype(np.float32))

